# revision 25
# baseline (speedup 1.0000x reference)
"""Trainium2 Bass kernel for nn_ChannelMix (segment_reduce / order-2 channel mix).

Problem: x (B=8, K=32, C=8, T=512) f32; weight (K, 36, C) is a *fixed* binary
combination-selector (rows = all C(8,1)+C(8,2) channel combinations in
itertools.combinations order, identical for every kernel k). Per (b, k, t):
  out[b, k, r, t]   = x[b, k, r, t]                      r in 0..7  (singles)
  out[b, k, 8+q, t] = x[b, k, i_q, t] * x[b, k, j_q, t]  pair q = (i_q, j_q)
(exact zeros would be replaced by 1.0 first; the seed-0 randn input has none,
and structural zeros are handled by only multiplying selected channels.)

Sharding: data-parallel over batch, one batch element per NeuronCore
(8 cores, SPMD, no collectives). weight never reaches the device.

The singles block is an identity copy of x, so it is assembled on the host
from the f32 input (exact); only the 28 pair-product rows ever touch the
device. Device I/O is fp16 (accuracy gate is 2e-2 relative; fp16 products
land ~1.5e-3), which halves both the input load and the pair writeback.

Host relayout makes every DMA a plain 2D partition-major copy:
  xin[u*32+k, c*128+v] = x[k, c, u*128+v]   (fp16 input,  128 x 1024)
  pout[u*32+k, q*128+v] = x_i*x_j           (fp16 output, 128 x 3584)
with pair blocks q in itertools.combinations order grouped by base channel.

Per-core schedule (all timings = TimelineSim cost model; the DMA device is
exclusive, so makespan ~= head + packed transfer stream + tail):
  loads  : channel ranges 4..8 / 2..4 (SP, HWDGE) + 0..2 (Pool, SWDGE),
           descending so high-channel muls start first; the load DMAs are
           hoisted into the framework preamble block so their ~1300ns issue
           latency overlaps the const-ap init + all-engine barrier
  DVE    : one tensor_mul per base channel c (in0 = channel-c block broadcast
           along the pair axis via stride-0 AP; in1 = channels c+1..7, fp16
           2x mode), order c = 6..0 so high pair blocks are ready first
  stores : 4 pair-block chunks (6/9/6/7 blocks on ACT/SP/ACT/SP), each
           gated on a dve_sem block count, sized so the exclusive DMA
           device never starves behind the ~630ns HWDGE + ~650/784ns DGE
           per-DMA issue latency; no explicit completion wait (the
           framework drain waits on the DMA semaphores)

Sync waits are standalone wait_ge instructions: fusing a wait onto a DMA
or tensor op via _wait_ge breaks this build's walrus codegen.
"""

import numpy as np

import concourse.bass as bass
from concourse import mybir
from concourse.bass_utils import run_bass_kernel_spmd

F16 = mybir.dt.float16
B, K, C, T = 8, 32, 8, 512
U, V = 4, 128  # t = u*V + v
N_CORES = 8
NPAIR = 28
# Pair blocks grouped by base channel: PBASE[c] = first block of channel c,
# blocks PBASE[c]..PBASE[c]+ND[c] are pairs (c, c+1)..(c, 7).
ND = [C - 1 - c for c in range(C)]  # 7,6,5,4,3,2,1,0
PBASE = [0]
for c in range(C - 1):
    PBASE.append(PBASE[-1] + ND[c])

# Schedule config (tunable): loads = (engine, c_lo, c_hi) channel ranges in
# issue order; muls = base-channel order on DVE; chunks = (engine, b_lo, b_hi)
# pair-block ranges in issue order per engine (thresholds derived);
# final_wait = explicit SP completion wait (the framework drain also waits
# for outstanding DMA sems, so this is belt-and-suspenders only).
CONFIG = dict(
    loads=[("sp", 4, 8), ("sp", 2, 4), ("pool", 0, 2)],
    muls=[6, 5, 4, 3, 2, 1, 0],
    chunks=[("act", 22, 28), ("sp", 13, 22), ("act", 7, 13), ("sp", 0, 7)],
    final_wait=False,
    fuse_waits=False,  # fused sem waits on DMA/mul insts break neuronxcc here
    monotonic_sem_count=0,
    hoist_loads=True,
)

# fp16 value scaling: x is scaled by SCALE on upload and pair products by
# SCALE^2 on download. Unscaled, products in (6e-8, 1e-6) land in the fp16
# subnormal range whose ~3e-8 grid error exceeds the 2e-2 gate against the
# 1e-6-clamped denominator; scaling moves them into the normal range.
SCALE = 8.0

_NC = None


def build_kernel(config=None):
    cfg = dict(CONFIG if config is None else config)
    loads = list(cfg["loads"])
    muls = list(cfg["muls"])
    chunks = list(cfg["chunks"])
    final_wait = cfg.get("final_wait", False)
    fuse = cfg.get("fuse_waits", True)
    fuse_muls = cfg.get("fuse_mul_waits", fuse)
    msc = cfg.get("monotonic_sem_count", 0)
    hoist = cfg.get("hoist_loads", True)

    # normalize mul entries: int c -> (c, c+1, 8); else (c, j_lo, j_hi)
    muls = [(m, m + 1, 8) if isinstance(m, int) else tuple(m) for m in muls]

    def piece_blocks(p):
        c, j0, j1 = p
        b0 = PBASE[c] + (j0 - c - 1)
        return b0, b0 + (j1 - j0)

    # dve_sem value after each mul piece in order (cumulative blocks)
    cum = []
    tot = 0
    for p in muls:
        tot += p[2] - p[1]
        cum.append(tot)
    assert tot == NPAIR, muls

    def chunk_thr(b_lo, b_hi):
        # threshold = cumulative dve_sem count through the last mul piece
        # whose block range intersects [b_lo, b_hi)
        need = 0
        for i, p in enumerate(muls):
            pb0, pb1 = piece_blocks(p)
            if pb0 < b_hi and pb1 > b_lo:
                need = max(need, cum[i])
        return need

    # which load chunks (by index) a mul piece requires: channels {c}+[j0,j1)
    def mul_loads(p):
        c, j0, j1 = p
        return [j for j, (_, lo, hi) in enumerate(loads)
                if (lo <= c < hi) or (lo < j1 and hi > j0)]

    nc = bass.Bass(monotonic_sem_count=msc)
    xin = nc.declare_dram_parameter("xin", [128, C * V], F16, isOutput=False)
    pout = nc.declare_dram_parameter("pout", [128, NPAIR * V], F16,
                                     isOutput=True)

    n_out = 16 * len(chunks)

    with (
        nc.sbuf_tensor([128, C * V], F16) as X,
        nc.sbuf_tensor([128, NPAIR * V], F16) as S,
        nc.semaphore("load0") as l0,
        nc.semaphore("load1") as l1,
        nc.semaphore("load2") as l2,
        nc.semaphore("dve_sem") as dve_sem,
        nc.semaphore("out_sem") as out_sem,
        nc.Block() as block,
    ):
        lsems = [l0, l1, l2][:len(loads)]
        assert len(loads) <= 3

        load_insts = []

        def emit(eng, which):
            for j, (e, lo, hi) in enumerate(loads):
                if e != which:
                    continue
                d = eng.dma_start(out=X[:, lo * V:hi * V],
                                  in_=xin[:, lo * V:hi * V]).then_inc(
                                      lsems[j], 16)
                load_insts.append(d.ins)
            for (e, b_lo, b_hi) in chunks:
                if e != which:
                    continue
                thr = chunk_thr(b_lo, b_hi)
                if not fuse:
                    eng.wait_ge(dve_sem, thr)
                d = eng.dma_start(out=pout[:, b_lo * V:b_hi * V],
                                  in_=S[:, b_lo * V:b_hi * V]
                                  ).then_inc(out_sem, 16)
                if fuse:
                    d._wait_ge(dve_sem, thr)

        @block.sync
        def _(sp):
            emit(sp, "sp")
            if final_wait:
                sp.wait_ge(out_sem, n_out)

        @block.scalar
        def _(act):
            emit(act, "act")

        @block.gpsimd
        def _(gp):
            emit(gp, "pool")

        @block.vector
        def _(v):
            waited = set()
            for p in muls:
                c, j0, j1 = p
                need = [j for j in mul_loads(p) if j not in waited]
                waited.update(need)
                if (fuse_muls and len(need) > 1) or (not fuse_muls and need):
                    # at most one fused wait per instruction on this build
                    for j in (need if not fuse_muls else need[:-1]):
                        v.wait_ge(lsems[j], 16)
                    need = need[-1:] if fuse_muls else []
                nd = j1 - j0
                b0 = PBASE[c] + (j0 - c - 1)
                in0 = X[:, c * V:(c + 1) * V].rearrange(
                    "p (one v) -> p one v", one=1).broadcast_to([128, nd, V])
                in1 = X[:, j0 * V:j1 * V].rearrange("p (d v) -> p d v", v=V)
                sv = S[:, b0 * V:(b0 + nd) * V].rearrange(
                    "p (d v) -> p d v", v=V)
                m = v.tensor_mul(sv, in0, in1).then_inc(dve_sem, nd)
                if fuse_muls and need:
                    m._wait_ge(lsems[need[0]], 16)

    if hoist:
        _hoist_loads(nc, load_insts)
    return nc


def _hoist_loads(nc, load_insts):
    """Move the input-load DMAs into the framework preamble block, just
    before their engine's preamble Drain (SP) / first const-ap Memset
    (Pool). The loads touch only the X SBUF region and carry their own
    semaphores, which are waited on after the all-engine barrier, so
    starting them before the barrier is safe; it buys ~650ns of issue
    latency that otherwise serializes behind the const-ap init."""
    fn = nc.m.functions[0]
    main = fn.blocks[0].instructions
    ids = {id(i) for i in load_insts}
    # remove from their body blocks
    for blk in fn.blocks[1:]:
        blk.instructions[:] = [i for i in blk.instructions
                               if id(i) not in ids]
    import concourse.mybir as mb
    inserted = {}
    for inst in load_insts:
        eng = inst.engine
        pos = None
        if eng == mb.EngineType.SP:
            # HWDGE DMAs don't read the preamble GPRs; issue before the
            # RegisterMoves so the descriptor-gen starts at t~25.
            pos = 1  # right after the dummy InstCall
        else:
            for k, mi in enumerate(main):
                if eng == mb.EngineType.Pool and isinstance(mi, mb.InstMemset):
                    pos = k
                    break
                if isinstance(mi, mb.InstDrain) and mi.engine == eng:
                    pos = k
                    break
        assert pos is not None, f"no hoist anchor for {eng}"
        # keep program order for multiple loads on the same engine
        pos = max(pos, inserted.get(eng, -1) + 1)
        main.insert(pos, inst)
        inserted[eng] = pos


def _get_nc():
    global _NC
    if _NC is None:
        _NC = build_kernel()
    return _NC


def _relayout_in(xb):
    # xb: (K, C, T) f32 -> (128, C*V) fp16 with p = u*32+k, col = c*V+v
    return np.ascontiguousarray(
        (xb * SCALE).reshape(K, C, U, V).transpose(2, 0, 1, 3).reshape(
            128, C * V)
    ).astype(np.float16)


def run(x, trace=False, **spmd_kwargs):
    x = np.ascontiguousarray(np.asarray(x), dtype=np.float32)
    assert x.shape == (B, K, C, T), x.shape
    in_maps = [{"xin": _relayout_in(x[b])} for b in range(B)]
    res = run_bass_kernel_spmd(_get_nc(), in_maps,
                               core_ids=list(range(N_CORES)),
                               trace=trace, **spmd_kwargs)
    out = np.empty((B, K, C + NPAIR, T), dtype=np.float32)
    out[:, :, 0:C, :] = x  # singles rows are an identity copy of the input
    inv = np.float32(1.0 / (SCALE * SCALE))
    for b in range(B):
        po = np.asarray(res.results[b]["pout"]).reshape(U, K, NPAIR, V)
        out[b, :, C:, :] = (
            po.transpose(1, 2, 0, 3).reshape(K, NPAIR, T).astype(np.float32)
            * inv)
    _fix_exact_zeros(x, out)
    return out, res


def _fix_exact_zeros(x, out):
    """Mirror the reference's where(y != 0, y, 1) zero replacement.

    An exact zero never occurs in the gaussian inputs this problem ships,
    so this is a no-op in practice; it exists so the kernel matches the
    reference contract on any input. A zero factor is replaced by 1.0
    before the product, so a zero single becomes 1.0 and a pair with one
    zero factor degenerates to the other factor.
    """
    if not (x == 0).any():
        return
    import itertools
    pairs = list(itertools.combinations(range(C), 2))
    zero = x == 0  # (B, K, C, T)
    out[:, :, 0:C, :] = np.where(zero, np.float32(1.0), x)
    for q, (i, j) in enumerate(pairs):
        r = C + q
        zi, zj = zero[:, :, i, :], zero[:, :, j, :]
        if not (zi.any() or zj.any()):
            continue
        xi, xj = x[:, :, i, :], x[:, :, j, :]
        fixed = np.where(zi & zj, np.float32(1.0),
                         np.where(zi, xj, np.where(zj, xi, out[:, :, r, :])))
        out[:, :, r, :] = fixed


def kernel(x, weight=None, **_unused):
    out, _ = run(x)
    return out


# revision 27
# speedup vs baseline: 1.0174x; 1.0174x over previous
"""Trainium2 Bass kernel for nn_ChannelMix (segment_reduce / order-2 channel mix).

Problem: x (B=8, K=32, C=8, T=512) f32; weight (K, 36, C) is a *fixed* binary
combination-selector (rows = all C(8,1)+C(8,2) channel combinations in
itertools.combinations order, identical for every kernel k). Per (b, k, t):
  out[b, k, r, t]   = x[b, k, r, t]                      r in 0..7  (singles)
  out[b, k, 8+q, t] = x[b, k, i_q, t] * x[b, k, j_q, t]  pair q = (i_q, j_q)
(exact zeros would be replaced by 1.0 first; handled on the host.)

Sharding: data-parallel over batch, one batch element per NeuronCore
(8 cores, SPMD, no collectives). weight never reaches the device.

The singles block is an identity copy of x, so it is assembled on the host
from the f32 input (exact); only the 28 pair-product rows ever touch the
device. Device I/O is fp16 (accuracy gate is 2e-2 relative; fp16 products
land ~2.8e-3), which halves both the input load and the pair writeback.
Inputs are scaled by SCALE on upload (and products unscaled on download) to
keep small products out of the fp16 subnormal range.

Host relayout makes every DMA a plain 2D partition-major copy:
  xin[u*32+k, c*128+v] = SCALE * x[k, c, u*128+v]   (fp16 in,  128 x 1024)
  pout[u*32+k, pos*128+v] = pair product            (fp16 out, 128 x 3584)
where pos is a permuted pair-block position (the pout/SBUF column layout is
the concatenation of the store chunks' block lists, so every store chunk and
every mul piece writes one contiguous column run).

Per-core schedule (timings = TimelineSim cost model; the DMA device is
exclusive, so makespan ~= head + packed transfer stream + tail):
  loads  : channel ranges 4..8 / 2..4 (SP, HWDGE) + 0..2 (Pool, SWDGE),
           hoisted into the framework preamble block so their ~1300ns issue
           latency overlaps the const-ap init + all-engine barrier
  compute: base-channel broadcast tensor_muls (in0 = channel-c block
           broadcast along the pair axis via stride-0 AP, fp16 2x mode on
           DVE); the two smallest groups (c6, c5) run on the otherwise-idle
           Pool engine in parallel so DVE starts directly on c4
  stores : 4 pair-block chunks gated on per-engine block-count semaphores,
           sized so the exclusive DMA device never starves behind the
           ~630ns HWDGE + ~650/784ns DGE per-DMA issue latency; no explicit
           completion wait (the framework drain waits on the DMA sems)

Sync waits are standalone wait_ge instructions: fusing a wait onto a DMA
or tensor op via _wait_ge breaks this build's walrus codegen.
"""

import numpy as np

import concourse.bass as bass
from concourse import mybir
from concourse.bass_utils import run_bass_kernel_spmd

F16 = mybir.dt.float16
B, K, C, T = 8, 32, 8, 512
U, V = 4, 128  # t = u*V + v
N_CORES = 8
NPAIR = 28
# Pair blocks grouped by base channel: PBASE[c] = first block of channel c,
# blocks PBASE[c]..PBASE[c]+ND[c] are pairs (c, c+1)..(c, 7).
ND = [C - 1 - c for c in range(C)]  # 7,6,5,4,3,2,1,0
PBASE = [0]
for c in range(C - 1):
    PBASE.append(PBASE[-1] + ND[c])

# Schedule config (tunable):
#   loads : (engine, c_lo, c_hi) channel ranges in issue order
#   muls  : (engine, c, j_lo, j_hi) pieces; int c / (c, j0, j1) = dve piece
#   chunks: (engine, blocks) where blocks is a block-id list or (b_lo, b_hi);
#           pout column layout = concatenation of the chunk block lists
CONFIG = dict(
    loads=[("sp", 4, 8), ("sp", 2, 4), ("pool", 0, 2)],
    muls=[("pool", 6, 7, 8),
          ("dve", 5, 6, 8), ("dve", 4, 5, 8), ("dve", 3, 4, 8),
          ("dve", 2, 3, 6), ("dve", 2, 6, 8),
          ("dve", 1, 2, 8), ("dve", 0, 1, 8)],
    chunks=[("act", [25, 26, 22, 23, 24, 27]),
            ("sp", [18, 19, 20, 21, 13, 14, 15]),
            ("sp", [16, 17, 7, 8, 9, 10, 11, 12]),
            ("sp", [0, 1, 2, 3, 4, 5, 6])],
    final_wait=False,
    fuse_waits=False,  # fused sem waits on DMA/mul insts break neuronxcc here
    monotonic_sem_count=0,
    hoist_loads=True,
)

# fp16 value scaling: x is scaled by SCALE on upload and pair products by
# SCALE^2 on download. Unscaled, products in (6e-8, 1e-6) land in the fp16
# subnormal range whose ~3e-8 grid error exceeds the 2e-2 gate against the
# 1e-6-clamped denominator; scaling moves them into the normal range.
SCALE = 8.0

_NC = None


def _norm_muls(muls):
    out = []
    for m in muls:
        if isinstance(m, int):
            out.append(("dve", m, m + 1, 8))
        elif len(m) == 3:
            out.append(("dve",) + tuple(m))
        else:
            out.append(tuple(m))
    return out


def _norm_chunks(chunks):
    out = []
    for (e, *rest) in chunks:
        if len(rest) == 2:
            out.append((e, list(range(rest[0], rest[1]))))
        else:
            out.append((e, list(rest[0])))
    return out


def _layout(cfg):
    """pout/SBUF pair-block column order = concat of chunk block lists."""
    lay = []
    for (_, blocks) in _norm_chunks(cfg["chunks"]):
        lay.extend(blocks)
    assert sorted(lay) == list(range(NPAIR)), lay
    return lay


def build_kernel(config=None):
    cfg = dict(CONFIG if config is None else config)
    loads = list(cfg["loads"])
    muls = _norm_muls(cfg["muls"])
    chunks = _norm_chunks(cfg["chunks"])
    final_wait = cfg.get("final_wait", False)
    fuse = cfg.get("fuse_waits", False)
    msc = cfg.get("monotonic_sem_count", 0)
    hoist = cfg.get("hoist_loads", True)

    layout = _layout(cfg)
    pos = {b: i for i, b in enumerate(layout)}

    def piece_blocks(p):
        _, c, j0, j1 = p
        b0 = PBASE[c] + (j0 - c - 1)
        return list(range(b0, b0 + (j1 - j0)))

    def piece_run(p):
        """Contiguous ascending column run of this piece in the layout."""
        ps = [pos[b] for b in piece_blocks(p)]
        assert ps == list(range(ps[0], ps[0] + len(ps))), (
            f"mul piece {p} blocks not contiguous in layout: {ps}")
        return ps[0], len(ps)

    # per-engine cumulative block counts after each piece, and per-block
    # owner (engine, cum-at-completion) for chunk thresholds
    cum = {"dve": 0, "pool": 0}
    block_done_at = {}
    for p in muls:
        e = p[0]
        n = p[3] - p[2]
        cum[e] += n
        for b in piece_blocks(p):
            block_done_at[b] = (e, cum[e])
    assert len(block_done_at) == NPAIR
    n_dve = cum["dve"]
    n_pool = cum["pool"]

    def chunk_thrs(blocks):
        th = {"dve": 0, "pool": 0}
        for b in blocks:
            e, k = block_done_at[b]
            th[e] = max(th[e], k)
        return th

    # which load chunks (by index) a mul piece requires: channels {c}+[j0,j1)
    def mul_loads(p):
        _, c, j0, j1 = p
        return [j for j, (_, lo, hi) in enumerate(loads)
                if (lo <= c < hi) or (lo < j1 and hi > j0)]

    nc = bass.Bass(monotonic_sem_count=msc)
    xin = nc.declare_dram_parameter("xin", [128, C * V], F16, isOutput=False)
    pout = nc.declare_dram_parameter("pout", [128, NPAIR * V], F16,
                                     isOutput=True)

    n_out = 16 * len(chunks)

    with (
        nc.sbuf_tensor([128, C * V], F16) as X,
        nc.sbuf_tensor([128, NPAIR * V], F16) as S,
        nc.semaphore("load0") as l0,
        nc.semaphore("load1") as l1,
        nc.semaphore("load2") as l2,
        nc.semaphore("dve_sem") as dve_sem,
        nc.semaphore("pool_sem") as pool_sem,
        nc.semaphore("out_sem") as out_sem,
        nc.Block() as block,
    ):
        csem = {"dve": dve_sem, "pool": pool_sem}
        lsems = [l0, l1, l2][:len(loads)]
        assert len(loads) <= 3

        load_insts = []

        def emit_loads(eng, which):
            for j, (e, lo, hi) in enumerate(loads):
                if e != which:
                    continue
                d = eng.dma_start(out=X[:, lo * V:hi * V],
                                  in_=xin[:, lo * V:hi * V]).then_inc(
                                      lsems[j], 16)
                load_insts.append(d.ins)

        def emit_chunks(eng, which):
            for (e, blocks) in chunks:
                if e != which:
                    continue
                th = chunk_thrs(blocks)
                # pool threshold usually fires first; wait it before dve
                if th["pool"]:
                    eng.wait_ge(pool_sem, th["pool"])
                if th["dve"]:
                    eng.wait_ge(dve_sem, th["dve"])
                p0 = pos[blocks[0]]
                assert [pos[b] for b in blocks] == list(
                    range(p0, p0 + len(blocks))), blocks
                eng.dma_start(out=pout[:, p0 * V:(p0 + len(blocks)) * V],
                              in_=S[:, p0 * V:(p0 + len(blocks)) * V]
                              ).then_inc(out_sem, 16)

        def emit_muls(v, which, waited):
            for p in muls:
                if p[0] != which:
                    continue
                for j in mul_loads(p):
                    if j not in waited:
                        v.wait_ge(lsems[j], 16)
                        waited.add(j)
                _, c, j0, j1 = p
                nd = j1 - j0
                c0, _n = piece_run(p)
                in0 = X[:, c * V:(c + 1) * V].rearrange(
                    "p (one v) -> p one v", one=1).broadcast_to([128, nd, V])
                in1 = X[:, j0 * V:j1 * V].rearrange("p (d v) -> p d v", v=V)
                sv = S[:, c0 * V:(c0 + nd) * V].rearrange(
                    "p (d v) -> p d v", v=V)
                v.tensor_mul(sv, in0, in1).then_inc(csem[which], nd)

        @block.sync
        def _(sp):
            emit_loads(sp, "sp")
            emit_chunks(sp, "sp")
            if final_wait:
                sp.wait_ge(out_sem, n_out)

        @block.scalar
        def _(act):
            emit_loads(act, "act")
            emit_chunks(act, "act")

        @block.gpsimd
        def _(gp):
            emit_loads(gp, "pool")
            emit_muls(gp, "pool", set())
            emit_chunks(gp, "pool")

        @block.vector
        def _(v):
            emit_muls(v, "dve", set())

    assert fuse is False  # fused waits unsupported on this build
    if hoist:
        _hoist_loads(nc, load_insts)
    return nc


def _hoist_loads(nc, load_insts):
    """Move the input-load DMAs into the framework preamble block, just
    before their engine's preamble Drain (SP) / first const-ap Memset
    (Pool). The loads touch only the X SBUF region and carry their own
    semaphores, which are waited on after the all-engine barrier, so
    starting them before the barrier is safe; it buys ~650ns of issue
    latency that otherwise serializes behind the const-ap init."""
    fn = nc.m.functions[0]
    main = fn.blocks[0].instructions
    ids = {id(i) for i in load_insts}
    # remove from their body blocks
    for blk in fn.blocks[1:]:
        blk.instructions[:] = [i for i in blk.instructions
                               if id(i) not in ids]
    import concourse.mybir as mb
    inserted = {}
    for inst in load_insts:
        eng = inst.engine
        pos_ = None
        if eng == mb.EngineType.SP:
            # HWDGE DMAs don't read the preamble GPRs; issue before the
            # RegisterMoves so the descriptor-gen starts at t~25.
            pos_ = 1  # right after the dummy InstCall
        else:
            for k, mi in enumerate(main):
                if eng == mb.EngineType.Pool and isinstance(mi, mb.InstMemset):
                    pos_ = k
                    break
                if isinstance(mi, mb.InstDrain) and mi.engine == eng:
                    pos_ = k
                    break
        assert pos_ is not None, f"no hoist anchor for {eng}"
        # keep program order for multiple loads on the same engine
        pos_ = max(pos_, inserted.get(eng, -1) + 1)
        main.insert(pos_, inst)
        inserted[eng] = pos_


def _get_nc():
    global _NC
    if _NC is None:
        _NC = build_kernel()
    return _NC


def _relayout_in(xb):
    # xb: (K, C, T) f32 -> (128, C*V) fp16 with p = u*32+k, col = c*V+v
    return np.ascontiguousarray(
        (xb * SCALE).reshape(K, C, U, V).transpose(2, 0, 1, 3).reshape(
            128, C * V)
    ).astype(np.float16)


def run(x, trace=False, **spmd_kwargs):
    x = np.ascontiguousarray(np.asarray(x), dtype=np.float32)
    assert x.shape == (B, K, C, T), x.shape
    in_maps = [{"xin": _relayout_in(x[b])} for b in range(B)]
    res = run_bass_kernel_spmd(_get_nc(), in_maps,
                               core_ids=list(range(N_CORES)),
                               trace=trace, **spmd_kwargs)
    out = np.empty((B, K, C + NPAIR, T), dtype=np.float32)
    out[:, :, 0:C, :] = x  # singles rows are an identity copy of the input
    inv = np.float32(1.0 / (SCALE * SCALE))
    rows = C + np.asarray(_layout(CONFIG))  # pout position -> output row
    for b in range(B):
        po = np.asarray(res.results[b]["pout"]).reshape(U, K, NPAIR, V)
        out[b, :, rows, :] = (
            po.transpose(2, 1, 0, 3).reshape(NPAIR, K, T).astype(np.float32)
            * inv)
    _fix_exact_zeros(x, out)
    return out, res


def _fix_exact_zeros(x, out):
    """Mirror the reference's where(y != 0, y, 1) zero replacement.

    An exact zero never occurs in the gaussian inputs this problem ships,
    so this is a no-op in practice; it exists so the kernel matches the
    reference contract on any input. A zero factor is replaced by 1.0
    before the product, so a zero single becomes 1.0 and a pair with one
    zero factor degenerates to the other factor.
    """
    if not (x == 0).any():
        return
    import itertools
    pairs = list(itertools.combinations(range(C), 2))
    zero = x == 0  # (B, K, C, T)
    out[:, :, 0:C, :] = np.where(zero, np.float32(1.0), x)
    for q, (i, j) in enumerate(pairs):
        r = C + q
        zi, zj = zero[:, :, i, :], zero[:, :, j, :]
        if not (zi.any() or zj.any()):
            continue
        xi, xj = x[:, :, i, :], x[:, :, j, :]
        fixed = np.where(zi & zj, np.float32(1.0),
                         np.where(zi, xj, np.where(zj, xi, out[:, :, r, :])))
        out[:, :, r, :] = fixed


def kernel(x, weight=None, **_unused):
    out, _ = run(x)
    return out
